# revision 23
# baseline (speedup 1.0000x reference)
"""Trainium2 Bass kernel for nn_CascadeGNN (cascade AGNN over 256 graphs).

Graph-sharded SPMD over 8 NeuronCores (32 data graphs/core + one packed
512-node query block). Dense per-graph AGNN with:
  * fp8 (e4m3) cos matmuls; the count-mask rides in PSUM as ln(ct)/beta
    accumulated via a DoubleRow matmul {identity, rank-1 c*ir x ir}, so
    exp(beta*PSUM) directly yields the masked edge weights W in fp8.
  * fp8 DoubleRow num/den matmuls (2 k-tiles of 128 src nodes each).
  * node-major h produced by extra N=128 matmuls (rz^T W2) instead of
    transposes; biases folded algebraically (b2 x den into the v-mask
    rank-1 term, so node-major h stays unbiased).
  * one activation-function table for the whole kernel (Ln/Exp/Identity/
    Relu all live in natural_log_exp_and_others; the act-table chooser is
    steered there to avoid per-pair table reloads).
  * software-pipelined emission: prep(pair p+1) is emitted before
    main(pair p) so every engine always has ready work queued.
"""

import threading
from contextlib import ExitStack

import numpy as np
import ml_dtypes

import concourse.bass as bass
import concourse.mybir as mybir
import concourse.tile as tile
from concourse import bacc
from concourse.bass import ds, ts
from concourse.bass_utils import run_bass_kernel_spmd
from concourse.hw_specs import get_activation_tables
from concourse.masks import make_identity

BF16 = mybir.dt.bfloat16
F32 = mybir.dt.float32
FP8 = mybir.dt.float8e4
AF = mybir.ActivationFunctionType
ALU = mybir.AluOpType
DR = mybir.MatmulPerfMode.DoubleRow

# problem constants
B = 256
NPG = 512
NQPG = 16
IN, H, L, OUT = 64, 128, 2, 1
NCORES = 8
GPC = B // NCORES          # graphs per core (32)
N = NPG                    # dense block size (512)
NCH = N // 128             # 4 chunks of 128 src nodes
G32 = N // NQPG            # 32 query graphs packed into one 512 block
LNZ = -16.0                # ln-count floor for absent edges
MASK_THR = 100.0           # recip threshold: real den -> recip < ~3

_ACT_KEEP = None


def _patched_tables(arch):
    """First-match table choice would thrash between 'natural_log' and
    'exp_and_others'; strip our funcs from earlier tables so every
    activation resolves to natural_log_exp_and_others (one load)."""
    tabs = get_activation_tables(arch)
    keep = {AF.Ln, AF.Exp, AF.Identity, AF.Relu, AF.Copy, AF.Square,
            AF.Sign, AF.MemsetZero, AF.Abs, AF.Is_finite}
    out = {}
    hit = False
    for name, s in tabs.items():
        if name == "natural_log_exp_and_others":
            out[name] = s
            hit = True
        elif not hit:
            out[name] = s - keep
        else:
            out[name] = s
    return out


def build_program(gpc=GPC):
    nc = bacc.Bacc("TRN2", target_bir_lowering=False, debug=False,
                   num_devices=NCORES)
    io = {}
    n_nodes = gpc * NPG
    io["xt"] = nc.dram_tensor("xt", [IN, n_nodes], BF16,
                              kind="ExternalInput").ap()
    io["xqt"] = nc.dram_tensor("xqt", [IN, N], BF16,
                               kind="ExternalInput").ap()
    io["ct"] = nc.dram_tensor("ct", [gpc, 5, 128, N], FP8,
                              kind="ExternalInput").ap()
    io["ctq"] = nc.dram_tensor("ctq", [5, 128, N], FP8,
                               kind="ExternalInput").ap()
    io["lhsb"] = nc.dram_tensor("lhsb", [gpc, 5, 128, 128], FP8,
                                kind="ExternalInput").ap()
    io["lhsbq"] = nc.dram_tensor("lhsbq", [5, 128, 128], FP8,
                                 kind="ExternalInput").ap()
    for nm, shp, dt in [
        ("wg", [IN, H], BF16), ("wq", [IN, H], BF16),
        ("bgc", [H, 1], F32), ("bqc", [H, 1], F32),
        ("bq2row", [L, 1, H], BF16),          # (2*bq, bq) per layer
        ("betg", [L, H, 1], F32), ("betq", [L, H, 1], F32),
        ("a1", [L, H, H], BF16), ("b1t", [L, H, H], BF16),
        ("w2", [L, H, H], BF16),
        ("b1c", [L, H, 1], F32), ("b2c", [L, H, 1], F32),
        ("bprev_bf", [L, H, 1], BF16),        # (bg, b2r[0]) per layer
        ("wp1", [H, H], BF16), ("wp2", [H, 1], BF16),
        ("bp1c", [H, 1], F32), ("bp2c", [1, 1], F32),
    ]:
        io[nm] = nc.dram_tensor(nm, shp, dt, kind="ExternalInput").ap()
    io["y"] = nc.dram_tensor("y", [1, gpc], F32, kind="ExternalOutput").ap()

    orig = bacc.get_activation_tables
    bacc.get_activation_tables = _patched_tables
    try:
        with tile.TileContext(nc) as tc:
            _emit(tc, nc, gpc, io)
        nc.compile()
    finally:
        bacc.get_activation_tables = orig
    return nc


def _emit(tc, nc, gpc, io):
    n_nodes = gpc * NPG
    NPAIR = gpc // 2

    ctx = ExitStack()
    with ctx:
        pconst = ctx.enter_context(tc.tile_pool(name="pconst", bufs=1))
        pstate = ctx.enter_context(tc.tile_pool(name="pstate", bufs=1))
        pwork = ctx.enter_context(tc.tile_pool(name="pwork", bufs=2))
        pwork3 = ctx.enter_context(tc.tile_pool(name="pwork3", bufs=3))
        pct = ctx.enter_context(tc.tile_pool(name="pct", bufs=2))
        prow = ctx.enter_context(tc.tile_pool(name="prow", bufs=2))
        prow1 = ctx.enter_context(tc.tile_pool(name="prow1", bufs=1))
        ps_cos = ctx.enter_context(
            tc.tile_pool(name="ps_cos", bufs=2, space="PSUM"))
        ps_acc = ctx.enter_context(
            tc.tile_pool(name="ps_acc", bufs=2, space="PSUM"))
        ps_row = ctx.enter_context(
            tc.tile_pool(name="ps_row", bufs=1, space="PSUM"))
        ps_qtp = ctx.enter_context(
            tc.tile_pool(name="ps_qtp", bufs=1, space="PSUM"))

        def const(name, shape, dtype):
            return pconst.tile(shape, dtype, name=name, tag=name)

        # ---- constants ----
        ident_bf = const("ident_bf", [128, 128], BF16)
        make_identity(nc, ident_bf[:])
        ones_col_bf = const("ones_col_bf", [128, 1], BF16)
        nc.vector.memset(ones_col_bf[:], 1.0)
        qeps = const("qeps", [1, 1], F32)
        nc.vector.memset(qeps[:], 1e-24)

        # ---- weights ----
        def load(name, ap_dram, shape, dtype):
            t = const(name, shape, dtype)
            nc.sync.dma_start(t[:], ap_dram)
            return t

        wg_s = load("wg_s", io["wg"][:], [IN, H], BF16)
        wq_s = load("wq_s", io["wq"][:], [IN, H], BF16)
        bgc_s = load("bgc_s", io["bgc"][:], [H, 1], F32)
        bqc_s = load("bqc_s", io["bqc"][:], [H, 1], F32)
        bq2row_s = [load(f"bq2row{l}", io["bq2row"][l], [1, H], BF16)
                    for l in range(L)]
        betg_s = [load(f"betg{l}", io["betg"][l], [H, 1], F32)
                  for l in range(L)]
        betq_s = [load(f"betq{l}", io["betq"][l], [H, 1], F32)
                  for l in range(L)]
        a1_s = [load(f"a1_{l}", io["a1"][l], [H, H], BF16) for l in range(L)]
        b1t_s = [load(f"b1t_{l}", io["b1t"][l], [H, H], BF16)
                 for l in range(L)]
        w2_s = [load(f"w2_{l}", io["w2"][l], [H, H], BF16) for l in range(L)]
        b1c_s = [load(f"b1c_{l}", io["b1c"][l], [H, 1], F32)
                 for l in range(L)]
        b2c_s = [load(f"b2c_{l}", io["b2c"][l], [H, 1], F32)
                 for l in range(L)]
        bprev_s = [load(f"bprev{l}", io["bprev_bf"][l], [H, 1], BF16)
                   for l in range(L)]
        wp1_s = load("wp1_s", io["wp1"][:], [H, H], BF16)
        wp2_s = load("wp2_s", io["wp2"][:], [H, 1], BF16)
        bp1c_s = load("bp1c_s", io["bp1c"][:], [H, 1], F32)
        bp2c_s = load("bp2c_s", io["bp2c"][:], [1, 1], F32)
        xqt_s = load("xqt_s", io["xqt"][:], [IN, N], BF16)
        ctq_t = load("ctq_t", io["ctq"][:].rearrange("b p f -> p b f"),
                     [128, 5, N], FP8)
        lhsbq_t = load("lhsbq_t",
                       io["lhsbq"][:].rearrange("b p f -> p b f"),
                       [128, 5, 128], FP8)
        lhsb_t = load("lhsb_t",
                      io["lhsb"][:].rearrange("g b p f -> p (g b) f"),
                      [128, gpc * 5, 128], FP8)

        # ---- persistent state ----
        def state(name, shape, dtype):
            return pstate.tile(shape, dtype, name=name, tag=name)

        HGT = state("HGT", [128, n_nodes], BF16)          # h_g feature-major
        HGN = state("HGN", [128, gpc * 4, 128], BF16)     # node-major, no b2
        HQT = state("HQT", [128, N], BF16)
        HQN = state("HQN", [128, 4, 128], BF16)
        HGS = state("HGS", [128, gpc], F32)
        u_bf = state("u_bf", [128, G32], BF16)
        c_row = state("c_row", [1, G32], F32)
        ce_row = state("ce_row", [1, G32], F32)
        vt32 = state("vt32", [G32, 128], BF16)
        vrow = state("vrow", [1, G32 * 128], BF16)
        wfold = state("wfold", [128, 1], F32)
        den_bfq = state("den_bfq", [1, N], BF16)

        def lb_g(g):
            return lhsb_t[:, ds(g * 5, 5), :]

        # ================= helpers =================
        def proj_graph(g):
            """Initial projection for data graph g: feature-major (bias bg
            via ACT) + node-major (no bias) into HGN8."""
            xg = pwork3.tile([IN, N], BF16, name="xg", tag="xg")
            nc.sync.dma_start(xg[:], io["xt"][:, ts(g, N)])
            fm = ps_acc.tile([128, N], F32, name="fm", tag="acc")
            nc.tensor.matmul(fm[:], wg_s[:], xg[:], start=True, stop=True)
            nc.scalar.activation(HGT[:, ts(g, N)], fm[:], AF.Identity,
                                 bias=bgc_s[:])
            nm = ps_acc.tile([128, N], F32, name="nm", tag="acc")
            for c in range(NCH):
                nc.tensor.matmul(nm[:, ts(c, 128)], xg[:, ts(c, 128)],
                                 wg_s[:], start=True, stop=True)
            nc.vector.tensor_copy(HGN[:, ds(g * 4, 4), :],
                                  nm[:].rearrange("p (c f) -> p c f", c=4))

        def attention(hn_tile, hn_off, lbt, lb_off, ctt, ct_off, hn8_lhs,
                      beta, wt8, num_ps, den_ps, num_stop):
            """Shared dense-AGNN core for one 512-node block.
            hn_tile[:, hn_off:hn_off+N]: fp8 normalized features.
            lbt/ctt: lhsT-B and ct tiles; blocks at lb_off/ct_off.
            hn8_lhs: node-major fp8 [128, 4, 128] lhsT for num.
            Emits cos+exp into wt8 [128,4,N], then num/den matmuls."""
            for half in range(2):
                cos = ps_cos.tile([128, 2 * N], F32, name="cos", tag="cos")
                for ci in range(2):
                    c = 2 * half + ci
                    sl = cos[:, ds(ci * N, N)]
                    nc.tensor.matmul(sl,
                                     hn_tile[:, ds(hn_off + c * 128, 128)],
                                     hn_tile[:, ds(hn_off, N)],
                                     start=True, stop=False,
                                     skip_group_check=True)
                    nc.tensor.matmul(
                        sl,
                        lbt[:, lb_off:(lb_off + 2 + c):(1 + c), :],
                        ctt[:, (ct_off + c):(ct_off + 5):(4 - c), :],
                        start=False, stop=True, perf_mode=DR,
                        skip_group_check=True)
                nc.scalar.activation(
                    wt8[:, ds(half * 2, 2), :],
                    cos[:].rearrange("p (c f) -> p c f", c=2),
                    AF.Exp, scale=beta)
            for c in range(NCH):
                nc.tensor.matmul(num_ps[:], hn8_lhs[:, c, :],
                                 wt8[:, c, :], start=(c == 0),
                                 stop=(num_stop and c == NCH - 1),
                                 skip_group_check=True)
                nc.tensor.matmul(den_ps[0:1, :], ones_col_bf[:],
                                 wt8[:, c, :], start=(c == 0),
                                 stop=(c == NCH - 1),
                                 skip_group_check=True)

        def norm_rows(den_ps, nn):
            """recip (DVE), mask (Pool, from recip), dmm (DVE), dmm bcast
            (Pool). Returns (dmp [128,nn] f32, mask_row bf16)."""
            rr = prow.tile([1, nn], F32, name="rr", tag="rr")
            nc.vector.reciprocal(rr[:], den_ps[0:1, :])
            mk = prow.tile([1, nn], BF16, name="mk", tag="mk")
            nc.gpsimd.tensor_scalar(mk[:], rr[:], MASK_THR, None,
                                    op0=ALU.is_lt)
            dm = prow.tile([1, nn], F32, name="dm", tag="dm")
            nc.vector.tensor_tensor(dm[:], rr[:], mk[:], op=ALU.mult)
            dmp = pwork.tile([128, nn], F32, name="dmp", tag="dmp")
            nc.gpsimd.partition_broadcast(dmp[:], dm[:])
            return dmp, mk

        # ================= query phase =================
        def query_phase(l):
            if l == 0:
                fm = ps_acc.tile([128, N], F32, name="fm", tag="acc")
                nc.tensor.matmul(fm[:], wq_s[:], xqt_s[:],
                                 start=True, stop=True)
                nc.scalar.activation(HQT[:], fm[:], AF.Identity,
                                     bias=bqc_s[:])
                nm = ps_acc.tile([128, N], F32, name="nm", tag="acc")
                for c in range(NCH):
                    nc.tensor.matmul(nm[:, ts(c, 128)], xqt_s[:, ts(c, 128)],
                                     wq_s[:], start=True, stop=True)
                nc.vector.tensor_copy(
                    HQN[:], nm[:].rearrange("p (c f) -> p c f", c=4))

            sqq = pwork.tile([128, N], BF16, name="sqq", tag="sqq")
            nc.vector.tensor_tensor(sqq[:], HQT[:], HQT[:], op=ALU.mult)
            nsq = ps_row.tile([32, N], F32, name="nsq", tag="row")
            nc.tensor.matmul(nsq[0:1, :], ones_col_bf[:], sqq[:],
                             start=True, stop=True)
            lnr = prow1.tile([1, N], F32, name="lnrq", tag="lnrq")
            nc.scalar.activation(lnr[:], nsq[0:1, :], AF.Ln, bias=qeps[:])
            irq = prow1.tile([1, N], BF16, name="irq", tag="irq")
            nc.scalar.activation(irq[:], lnr[:], AF.Exp, scale=-0.5)
            irb = pwork.tile([128, N], BF16, name="irbq", tag="irbq")
            nc.gpsimd.partition_broadcast(irb[:], irq[:])
            hnq = pwork.tile([128, N], FP8, name="hnq", tag="hnq")
            nc.vector.tensor_tensor(hnq[:], HQT[:], irb[:], op=ALU.mult)

            wtq = pwork.tile([128, 4, N], BF16, name="wtq", tag="wtq")
            num_ps = ps_acc.tile([128, N], F32, name="numq", tag="acc")
            den_ps = ps_row.tile([32, N], F32, name="denq", tag="row")
            attention(hnq, 0, lhsbq_t, 0, ctq_t, 0, HQN[:],
                      betq_s[l][:], wtq[:], num_ps, den_ps, num_stop=False)
            # query bias fold: num += k*bq (x) den  (k=2 at l=0, 1 later)
            nc.vector.tensor_copy(den_bfq[:], den_ps[0:1, :])
            nc.tensor.matmul(num_ps[:], bq2row_s[l][:], den_bfq[:],
                             start=False, stop=True, skip_group_check=True)
            dmp, _mk = norm_rows(den_ps, N)
            nc.vector.tensor_tensor(HQT[:], num_ps[:], dmp[:], op=ALU.mult)
            if l < L - 1:
                tp = ps_qtp.tile([128, 4, 128], BF16, name="qtp", tag="qtp")
                for c in range(NCH):
                    nc.tensor.transpose(tp[:, c, :], HQT[:, ts(c, 128)],
                                        ident_bf[:])
                nc.vector.tensor_copy(HQN[:], tp[:])

            # per-graph aggregates u, c, v
            uf = pwork.tile([128, G32], F32, name="uf", tag="uf")
            nc.vector.tensor_reduce(
                uf[:], HQT[:].rearrange("p (g k) -> p g k", k=NQPG),
                axis=mybir.AxisListType.X, op=ALU.add)
            nc.vector.tensor_copy(u_bf[:], uf[:])
            squ = pwork.tile([128, G32], BF16, name="squ", tag="uf")
            nc.vector.tensor_tensor(squ[:], uf[:], uf[:], op=ALU.mult)
            crow_ps = ps_row.tile([32, N], F32, name="crow_ps", tag="row")
            nc.tensor.matmul(crow_ps[0:1, 0:G32], ones_col_bf[:], squ[:],
                             start=True, stop=True)
            nc.vector.tensor_copy(c_row[:], crow_ps[0:1, 0:G32])
            nc.vector.tensor_scalar(ce_row[:], crow_ps[0:1, 0:G32], 1e-24,
                                    None, op0=ALU.add)
            # w = b1t^T u + a1^T b_prev  (the v-mask rank-1 vector)
            wv = ps_acc.tile([128, N], F32, name="wv", tag="acc")
            nc.tensor.matmul(wv[:, 0:G32], b1t_s[l][:], u_bf[:],
                             start=True, stop=True, skip_group_check=True)
            nc.tensor.matmul(wv[:, ds(G32, 1)], a1_s[l][:], bprev_s[l][:],
                             start=True, stop=True, skip_group_check=True)
            nc.vector.tensor_copy(wfold[:], wv[:, ds(G32, 1)])
            wsb = pwork.tile([128, G32], BF16, name="wsb", tag="uf")
            nc.scalar.activation(wsb[:], wv[:, 0:G32], AF.Identity,
                                 bias=wfold[:])
            vt_ps = ps_qtp.tile([128, 4, 128], BF16, name="vtp", tag="qtp")
            nc.tensor.transpose(vt_ps[0:G32, 0, :], wsb[:], ident_bf[:])
            nc.vector.tensor_copy(vt32[:], vt_ps[0:G32, 0, :])
            nc.sync.dma_start(vrow[:], vt32[:])

        # ================= data pipeline =================
        def prep(l, p):
            gA = 2 * p
            if l == 0:
                for gi in range(2):
                    proj_graph(gA + gi)
            ctp = pct.tile([128, 10, N], FP8, name="ctp", tag="ctp")
            nc.sync.dma_start(
                ctp[:],
                io["ct"][ds(gA, 2)].rearrange("g b p f -> p (g b) f"))
            hTp = HGT[:, ds(gA * N, 2 * N)]
            sq = pwork.tile([128, 2 * N], BF16, name="sq", tag="sq")
            nc.vector.tensor_tensor(sq[:], hTp, hTp, op=ALU.mult)
            lnr = prow.tile([1, 2 * N], F32, name="lnr", tag="lnr")
            for gi in range(2):
                nsq = ps_row.tile([32, N], F32, name="nsq", tag="row")
                nc.tensor.matmul(nsq[0:1, :], ones_col_bf[:],
                                 sq[:, ds(gi * N, N)], start=True, stop=True)
                nc.scalar.activation(lnr[0:1, ds(gi * N, N)], nsq[0:1, :],
                                     AF.Ln,
                                     bias=ce_row[0:1, ds(gA + gi, 1)])
            irr = prow.tile([1, 2 * N], BF16, name="irr", tag="irr")
            nc.scalar.activation(irr[:], lnr[:], AF.Exp, scale=-0.5)
            # b-rows (ir, fp8) into the ct b-blocks of both graphs
            nc.gpsimd.tensor_scalar(
                ctp[0:1, 4:10:5, :],
                irr[:].rearrange("o (g f) -> o g f", g=2), 1.0, None,
                op0=ALU.mult)
            # a-rows (c*ir, fp8) into lhsb blocks 1..4 of each graph
            for gi in range(2):
                g = gA + gi
                nc.vector.tensor_scalar(
                    lhsb_t[0:1, ds(g * 5 + 1, 4), :],
                    irr[0:1, ds(gi * N, N)].rearrange(
                        "o (c f) -> o c f", c=4),
                    c_row[0:1, ds(g, 1)], None, op0=ALU.mult)
            irb = pwork.tile([128, 2 * N], BF16, name="irb", tag="irb")
            nc.gpsimd.partition_broadcast(irb[:], irr[:])
            hn = pwork.tile([128, 2 * N], FP8, name="hn", tag="hn")
            nc.vector.tensor_tensor(hn[:], hTp, irb[:], op=ALU.mult)
            return hn, ctp

        def main(l, p, hn, ctp):
            gA = 2 * p
            for gi in range(2):
                g = gA + gi
                wt8 = pwork.tile([128, 4, N], BF16, name="wt8", tag="wt8")
                num_ps = ps_acc.tile([128, N], F32, name="num", tag="acc")
                den_ps = ps_row.tile([32, N], F32, name="den", tag="row")
                attention(hn, gi * N, lhsb_t, g * 5, ctp, gi * 5,
                          HGN[:, ds(g * 4, 4), :], betg_s[l][:], wt8[:],
                          num_ps, den_ps, num_stop=True)
                dmp, mk = norm_rows(den_ps, N)
                s1 = pwork.tile([128, N], BF16, name="s1", tag="s1")
                nc.vector.tensor_tensor(s1[:], num_ps[:], dmp[:],
                                        op=ALU.mult)
                z_ps = ps_acc.tile([128, N], F32, name="z", tag="acc")
                nc.tensor.matmul(z_ps[:], a1_s[l][:], s1[:],
                                 start=True, stop=False,
                                 skip_group_check=True)
                nc.tensor.matmul(z_ps[:], vrow[0:1, ts(g, 128)], mk[:],
                                 start=False, stop=True,
                                 skip_group_check=True)
                rz = pwork.tile([128, N], BF16, name="rz", tag="s1")
                nc.vector.tensor_scalar(rz[:], z_ps[:], b1c_s[l][:], 0.0,
                                        op0=ALU.add, op1=ALU.max)
                h2_ps = ps_acc.tile([128, N], F32, name="h2", tag="acc")
                nc.tensor.matmul(h2_ps[:], w2_s[l][:], rz[:],
                                 start=True, stop=True)
                nc.scalar.activation(HGT[:, ts(g, N)], h2_ps[:],
                                     AF.Identity, bias=b2c_s[l][:])
                if l < L - 1:
                    nm_ps = ps_acc.tile([128, N], F32, name="nm", tag="acc")
                    for c in range(NCH):
                        nc.tensor.matmul(nm_ps[:, ts(c, 128)],
                                         rz[:, ts(c, 128)], w2_s[l][:],
                                         start=True, stop=True)
                    nc.vector.tensor_copy(
                        HGN[:, ds(g * 4, 4), :],
                        nm_ps[:].rearrange("p (c f) -> p c f", c=4))
                else:
                    nc.vector.tensor_reduce(
                        HGS[:, ds(g, 1)], HGT[:, ts(g, N)],
                        axis=mybir.AxisListType.X, op=ALU.add)

        # ================= schedule =================
        for l in range(L):
            query_phase(l)
            carry = prep(l, 0)
            for p in range(NPAIR):
                nxt = prep(l, p + 1) if p + 1 < NPAIR else None
                main(l, p, *carry)
                carry = nxt

        # ---- final predictor ----
        hgs_bf = pwork.tile([128, G32], BF16, name="hgs_bf", tag="uf")
        nc.vector.tensor_copy(hgs_bf[:], HGS[:])
        z1 = ps_acc.tile([128, N], F32, name="z1", tag="acc")
        nc.tensor.matmul(z1[:, 0:G32], wp1_s[:], hgs_bf[:],
                         start=True, stop=True, skip_group_check=True)
        r1 = pwork.tile([128, G32], BF16, name="r1", tag="uf")
        nc.scalar.activation(r1[:], z1[:, 0:G32], AF.Relu, bias=bp1c_s[:])
        y_ps = ps_row.tile([32, N], F32, name="y_ps", tag="row")
        nc.tensor.matmul(y_ps[0:1, 0:G32], wp2_s[:], r1[:],
                         start=True, stop=True)
        y_sb = prow1.tile([1, G32], F32, name="y_sb", tag="ysb")
        nc.scalar.activation(y_sb[:], y_ps[0:1, 0:G32], AF.Identity,
                             bias=bp2c_s[:])
        nc.sync.dma_start(io["y"][:], y_sb[:])


# ================= host side =================

def _build_ct_np(src, dst, npb, nblocks):
    blk = src // npb
    s = src - blk * npb
    d = dst - blk * npb
    flat = blk * (npb * npb) + s * npb + d
    cnt = np.bincount(flat, minlength=nblocks * npb * npb)
    return cnt.reshape(nblocks, npb, npb)


def _lnct(counts, beta):
    """[nb, 512, 512] counts -> [nb, 5, 128, 512] fp8 ln(ct)/beta with
    LNZ floor, blocks 0..3 = src chunks, block 4 = zeros (b-block)."""
    nb = counts.shape[0]
    out = np.full((nb, 5, 128, N), 0.0, np.float32)
    with np.errstate(divide="ignore"):
        lv = np.where(counts > 0, np.log(np.maximum(counts, 1)),
                      LNZ * beta).astype(np.float32) / beta
    out[:, 0:4] = lv.reshape(nb, 4, 128, N)
    out[:, 4] = 0.0
    return out.astype(ml_dtypes.float8_e4m3)


_PROG_CACHE = {}
_PROG_LOCK = threading.Lock()


def _get_program(gpc=GPC):
    with _PROG_LOCK:
        if gpc not in _PROG_CACHE:
            _PROG_CACHE[gpc] = build_program(gpc)
        return _PROG_CACHE[gpc]


def _make_in_maps(inputs, gpc=GPC, ncores=NCORES):
    bf = ml_dtypes.bfloat16
    f8 = ml_dtypes.float8_e4m3
    X = np.asarray(inputs["X"], np.float32)
    X_q = np.asarray(inputs["X_q"], np.float32)
    g_src = np.asarray(inputs["g_src"], np.int64)
    g_dst = np.asarray(inputs["g_dst"], np.int64)
    q_src = np.asarray(inputs["q_src"], np.int64)
    q_dst = np.asarray(inputs["q_dst"], np.int64)
    betas_g = np.asarray(inputs["betas_g"], np.float32)
    betas_q = np.asarray(inputs["betas_q"], np.float32)
    assert np.all(betas_g > 0) and np.all(betas_q > 0)
    assert np.allclose(betas_g, betas_g[0]) and np.allclose(betas_q,
                                                            betas_q[0])

    W1r = np.asarray(inputs["W1r"], np.float32)
    bg = np.asarray(inputs["bg"], np.float32)
    bq = np.asarray(inputs["bq"], np.float32)
    b1r = np.asarray(inputs["b1r"], np.float32)
    b2r = np.asarray(inputs["b2r"], np.float32)
    bprev = np.stack([bg, b2r[0]])
    bq2 = np.stack([2.0 * bq, bq]).reshape(L, 1, H)

    # lhsb constant part: per graph block0 = identity, blocks 1..4 zero
    lhsb_one = np.zeros((5, 128, 128), np.float32)
    lhsb_one[0] = np.eye(128, dtype=np.float32)
    lhsb = np.broadcast_to(lhsb_one, (gpc, 5, 128, 128)).astype(f8)
    lhsbq = lhsb_one.astype(f8)

    shared = {
        "wg": np.asarray(inputs["Wg"], np.float32).astype(bf),
        "wq": np.asarray(inputs["Wq"], np.float32).astype(bf),
        "bgc": bg.reshape(H, 1).copy(),
        "bqc": bq.reshape(H, 1).copy(),
        "bq2row": bq2.astype(bf),
        "betg": np.tile(betas_g.reshape(L, 1, 1), (1, H, 1)),
        "betq": np.tile(betas_q.reshape(L, 1, 1), (1, H, 1)),
        "a1": np.ascontiguousarray(W1r[:, :H, :]).astype(bf),
        "b1t": np.ascontiguousarray(W1r[:, H:, :]).astype(bf),
        "w2": np.asarray(inputs["W2r"], np.float32).astype(bf),
        "b1c": b1r.reshape(L, H, 1).copy(),
        "b2c": b2r.reshape(L, H, 1).copy(),
        "bprev_bf": bprev.reshape(L, H, 1).astype(bf),
        "wp1": np.asarray(inputs["Wp1"], np.float32).astype(bf),
        "wp2": np.asarray(inputs["Wp2"], np.float32).astype(bf),
        "bp1c": np.asarray(inputs["bp1"], np.float32).reshape(H, 1).copy(),
        "bp2c": np.asarray(inputs["bp2"], np.float32).reshape(1, 1).copy(),
        "lhsb": lhsb,
        "lhsbq": lhsbq,
    }

    n = gpc * NPG
    nq = gpc * NQPG
    ne = n * 8
    nqe = nq * 8
    in_maps = []
    for cid in range(ncores):
        xc = X[cid * n:(cid + 1) * n]
        xqc = X_q[cid * nq:(cid + 1) * nq]
        gs = g_src[cid * ne:(cid + 1) * ne] - cid * n
        gd = g_dst[cid * ne:(cid + 1) * ne] - cid * n
        qs = q_src[cid * nqe:(cid + 1) * nqe] - cid * nq
        qd = q_dst[cid * nqe:(cid + 1) * nqe] - cid * nq

        ct_g_counts = _build_ct_np(gs, gd, NPG, gpc)     # [gpc, 512, 512]
        ct_q = _build_ct_np(qs, qd, NQPG, gpc)           # [gpc, 16, 16]
        ctq_blk = np.zeros((N, N), np.int64)
        for g in range(gpc):
            ctq_blk[g * NQPG:(g + 1) * NQPG,
                    g * NQPG:(g + 1) * NQPG] = ct_q[g]

        m = dict(shared)
        m["xt"] = np.ascontiguousarray(xc.T).astype(bf)
        xqt = np.zeros((IN, N), np.float32)
        xqt[:, :nq] = xqc.T
        m["xqt"] = xqt.astype(bf)
        m["ct"] = _lnct(ct_g_counts, float(betas_g[0]))
        m["ctq"] = _lnct(ctq_blk[None], float(betas_q[0]))[0]
        in_maps.append(m)
    return in_maps


def run(inputs, trace=False, gpc=GPC):
    nc = _get_program(gpc)
    in_maps = _make_in_maps(inputs, gpc=gpc)
    res = run_bass_kernel_spmd(nc, in_maps, list(range(NCORES)), trace=trace)
    ys = [res.results[c]["y"].reshape(-1) for c in range(NCORES)]
    out = np.concatenate(ys).astype(np.float32).reshape(B, OUT)
    return out, res


def kernel(**inputs) -> np.ndarray:
    out, _ = run(inputs, trace=False)
    return out


# revision 36
# speedup vs baseline: 1.2745x; 1.2745x over previous
"""Trainium2 Bass kernel for nn_CascadeGNN (cascade AGNN over 256 graphs).

Graph-sharded SPMD over 8 NeuronCores (32 data graphs/core + one packed
512-node query block). Dense per-graph AGNN with:
  * fp8 (e4m3) cos matmuls; the count-mask rides in PSUM as ln(ct)/beta
    accumulated via a DoubleRow matmul {identity, rank-1 c*ir x ir}, so
    exp(beta*PSUM) directly yields the masked edge weights W in fp8.
  * fp8 DoubleRow num/den matmuls (2 k-tiles of 128 src nodes each).
  * node-major h produced by extra N=128 matmuls (rz^T W2) instead of
    transposes; biases folded algebraically (b2 x den into the v-mask
    rank-1 term, so node-major h stays unbiased).
  * one activation-function table for the whole kernel (Ln/Exp/Identity/
    Relu all live in natural_log_exp_and_others; the act-table chooser is
    steered there to avoid per-pair table reloads).
  * software-pipelined emission: prep(pair p+1) is emitted before
    main(pair p) so every engine always has ready work queued.
"""

import threading
from contextlib import ExitStack

import numpy as np
import ml_dtypes

import concourse.bass as bass
import concourse.mybir as mybir
import concourse.tile as tile
from concourse import bacc
from concourse.bass import ds, ts
from concourse.bass_utils import run_bass_kernel_spmd
from concourse.hw_specs import get_activation_tables
from concourse.masks import make_identity

BF16 = mybir.dt.bfloat16
F32 = mybir.dt.float32
FP8 = mybir.dt.float8e4
AF = mybir.ActivationFunctionType
ALU = mybir.AluOpType
DR = mybir.MatmulPerfMode.DoubleRow

# problem constants
B = 256
NPG = 512
NQPG = 16
IN, H, L, OUT = 64, 128, 2, 1
NCORES = 8
GPC = B // NCORES          # graphs per core (32)
N = NPG                    # dense block size (512)
NCH = N // 128             # 4 chunks of 128 src nodes
G32 = N // NQPG            # 32 query graphs packed into one 512 block
LNZ = -16.0                # ln-count floor for absent edges
MASK_THR = 100.0           # recip threshold: real den -> recip < ~3

_ACT_KEEP = None


def _patched_tables(arch):
    """First-match table choice would thrash between 'natural_log' and
    'exp_and_others'; strip our funcs from earlier tables so every
    activation resolves to natural_log_exp_and_others (one load)."""
    tabs = get_activation_tables(arch)
    keep = {AF.Ln, AF.Exp, AF.Identity, AF.Relu, AF.Copy, AF.Square,
            AF.Sign, AF.MemsetZero, AF.Abs, AF.Is_finite}
    out = {}
    hit = False
    for name, s in tabs.items():
        if name == "natural_log_exp_and_others":
            out[name] = s
            hit = True
        elif not hit:
            out[name] = s - keep
        else:
            out[name] = s
    return out


def build_program(gpc=GPC):
    nc = bacc.Bacc("TRN2", target_bir_lowering=False, debug=False,
                   num_devices=NCORES)
    io = {}
    n_nodes = gpc * NPG
    io["xt"] = nc.dram_tensor("xt", [IN, n_nodes], BF16,
                              kind="ExternalInput").ap()
    io["xqt"] = nc.dram_tensor("xqt", [IN, N], BF16,
                               kind="ExternalInput").ap()
    io["ct"] = nc.dram_tensor("ct", [gpc, 5, 128, N], FP8,
                              kind="ExternalInput").ap()
    io["ctq"] = nc.dram_tensor("ctq", [5, 128, N], FP8,
                               kind="ExternalInput").ap()
    io["lhsb"] = nc.dram_tensor("lhsb", [gpc, 5, 128, 128], FP8,
                                kind="ExternalInput").ap()
    io["lhsbq"] = nc.dram_tensor("lhsbq", [5, 128, 128], FP8,
                                 kind="ExternalInput").ap()
    for nm, shp, dt in [
        ("wg", [IN, H], BF16), ("wq", [IN, H], BF16),
        ("bgc", [H, 1], F32), ("bqc", [H, 1], F32),
        ("bq2row", [L, 1, H], BF16),          # (2*bq, bq) per layer
        ("betg", [L, H, 1], F32), ("betq", [L, H, 1], F32),
        ("a1", [L, H, H], BF16), ("b1t", [L, H, H], BF16),
        ("w2", [L, H, H], BF16),
        ("b1c", [L, H, 1], F32), ("b2c", [L, H, 1], F32),
        ("bprev_bf", [L, H, 1], BF16),        # (bg, b2r[0]) per layer
        ("wp1", [H, H], BF16), ("wp2", [H, 1], BF16),
        ("b2x512c", [H, 1], F32),
        ("bp1c", [H, 1], F32), ("bp2c", [1, 1], F32),
    ]:
        io[nm] = nc.dram_tensor(nm, shp, dt, kind="ExternalInput").ap()
    io["y"] = nc.dram_tensor("y", [1, gpc], F32, kind="ExternalOutput").ap()

    orig = bacc.get_activation_tables
    bacc.get_activation_tables = _patched_tables
    try:
        with tile.TileContext(nc) as tc:
            _emit(tc, nc, gpc, io)
        nc.compile()
    finally:
        bacc.get_activation_tables = orig
    return nc


def _emit(tc, nc, gpc, io):
    n_nodes = gpc * NPG
    NPAIR = gpc // 2

    ctx = ExitStack()
    with ctx:
        pconst = ctx.enter_context(tc.tile_pool(name="pconst", bufs=1))
        pstate = ctx.enter_context(tc.tile_pool(name="pstate", bufs=1))
        pwork = ctx.enter_context(tc.tile_pool(name="pwork", bufs=2))
        pct = ctx.enter_context(tc.tile_pool(name="pct", bufs=3))
        prow = ctx.enter_context(tc.tile_pool(name="prow", bufs=2))
        prow1 = ctx.enter_context(tc.tile_pool(name="prow1", bufs=1))
        pring = ctx.enter_context(tc.tile_pool(name="pring", bufs=4))
        pring2 = ctx.enter_context(tc.tile_pool(name="pring2", bufs=2))
        pmid = ctx.enter_context(tc.tile_pool(name="pmid", bufs=3))
        ps_cos = ctx.enter_context(
            tc.tile_pool(name="ps_cos", bufs=3, space="PSUM"))
        ps_acc = ctx.enter_context(
            tc.tile_pool(name="ps_acc", bufs=3, space="PSUM"))
        ps_row = ctx.enter_context(
            tc.tile_pool(name="ps_row", bufs=1, space="PSUM"))

        def const(name, shape, dtype):
            return pconst.tile(shape, dtype, name=name, tag=name)

        # ---- constants ----
        ident_f = const("ident_f", [128, 128], F32)
        make_identity(nc, ident_f[:])
        ones_col_bf = const("ones_col_bf", [128, 1], BF16)
        nc.vector.memset(ones_col_bf[:], 1.0)
        qeps = const("qeps", [1, 1], F32)
        nc.vector.memset(qeps[:], 1e-24)

        # ---- weights ----
        def load(name, ap_dram, shape, dtype):
            t = const(name, shape, dtype)
            nc.sync.dma_start(t[:], ap_dram)
            return t

        wg_s = load("wg_s", io["wg"][:], [IN, H], BF16)
        wq_s = load("wq_s", io["wq"][:], [IN, H], BF16)
        bgc_s = load("bgc_s", io["bgc"][:], [H, 1], F32)
        bqc_s = load("bqc_s", io["bqc"][:], [H, 1], F32)
        bq2row_s = [load(f"bq2row{l}", io["bq2row"][l], [1, H], BF16)
                    for l in range(L)]
        betg_s = [load(f"betg{l}", io["betg"][l], [H, 1], F32)
                  for l in range(L)]
        betq_s = [load(f"betq{l}", io["betq"][l], [H, 1], F32)
                  for l in range(L)]
        a1_s = [load(f"a1_{l}", io["a1"][l], [H, H], BF16) for l in range(L)]
        b1t_s = [load(f"b1t_{l}", io["b1t"][l], [H, H], BF16)
                 for l in range(L)]
        w2_s = [load(f"w2_{l}", io["w2"][l], [H, H], BF16) for l in range(L)]
        b1c_s = [load(f"b1c_{l}", io["b1c"][l], [H, 1], F32)
                 for l in range(L)]
        b2c_s = [load(f"b2c_{l}", io["b2c"][l], [H, 1], F32)
                 for l in range(L)]
        bprev_s = [load(f"bprev{l}", io["bprev_bf"][l], [H, 1], BF16)
                   for l in range(L)]
        wp1_s = load("wp1_s", io["wp1"][:], [H, H], BF16)
        wp2_s = load("wp2_s", io["wp2"][:], [H, 1], BF16)
        bp1c_s = load("bp1c_s", io["bp1c"][:], [H, 1], F32)
        b2x512_s = load("b2x512_s", io["b2x512c"][:], [H, 1], F32)
        bp2c_s = load("bp2c_s", io["bp2c"][:], [1, 1], F32)
        xqt_s = load("xqt_s", io["xqt"][:], [IN, N], BF16)
        ctq_t = load("ctq_t", io["ctq"][:].rearrange("b p f -> p b f"),
                     [128, 5, N], FP8)
        lhsbq_t = load("lhsbq_t",
                       io["lhsbq"][:].rearrange("b p f -> p b f"),
                       [128, 5, 128], FP8)
        lhsb_t = load("lhsb_t",
                      io["lhsb"][:].rearrange("g b p f -> p (g b) f"),
                      [128, gpc * 5, 128], FP8)

        # ---- persistent state ----
        def state(name, shape, dtype):
            return pstate.tile(shape, dtype, name=name, tag=name)

        HGT = state("HGT", [128, n_nodes], BF16)          # h_g feature-major
        HGN = state("HGN", [128, gpc * 4, 128], BF16)     # node-major, no b2
        HQT = state("HQT", [128, N], F32)
        HQN = state("HQN", [128, 4, 128], BF16)
        HGS = state("HGS", [128, gpc], F32)
        u_bf = state("u_bf", [128, G32], BF16)
        c_row = state("c_row", [1, G32], F32)
        ce_row = state("ce_row", [1, G32], F32)
        vt32 = state("vt32", [G32, 128], BF16)
        vrow = state("vrow", [1, G32 * 128], BF16)
        wfold = state("wfold", [128, 1], F32)
        den_bfq = state("den_bfq", [1, N], BF16)

        def lb_g(g):
            return lhsb_t[:, ds(g * 5, 5), :]

        # ================= helpers =================
        def proj_graph(g, xgp, gi):
            """Initial projection for data graph g: feature-major (bias bg
            via ACT) + node-major (no bias) into HGN. xgp: [IN, 2N] pair."""
            fm = ps_acc.tile([128, N], F32, name="fm", tag="acc")
            nc.tensor.matmul(fm[:], wg_s[:], xgp[:, ds(gi * N, N)],
                             start=True, stop=True)
            nc.scalar.activation(HGT[:, ts(g, N)], fm[:], AF.Identity,
                                 bias=bgc_s[:])
            nm = ps_acc.tile([128, N], F32, name="nm", tag="acc")
            for c in range(NCH):
                nc.tensor.matmul(nm[:, ts(c, 128)],
                                 xgp[:, ds(gi * N + c * 128, 128)],
                                 wg_s[:], start=True, stop=True)
            nc.vector.tensor_copy(HGN[:, ds(g * 4, 4), :],
                                  nm[:].rearrange("p (c f) -> p c f", c=4))

        def attention(hn_tile, hn_off, lbt, lb_off, ctt, ct_off, hn8_lhs,
                      beta, wt8, num_ps, den_ps, num_stop):
            """Shared dense-AGNN core for one 512-node block.
            hn_tile[:, hn_off:hn_off+N]: fp8 normalized features.
            lbt/ctt: lhsT-B and ct tiles; blocks at lb_off/ct_off.
            hn8_lhs: node-major fp8 [128, 4, 128] lhsT for num.
            Emits cos+exp into wt8 [128,4,N], then num/den matmuls."""
            for c in range(NCH):
                cos = ps_cos.tile([128, N], F32, name="cos", tag="cos")
                nc.tensor.matmul(cos[:],
                                 hn_tile[:, ds(hn_off + c * 128, 128)],
                                 hn_tile[:, ds(hn_off, N)],
                                 start=True, stop=False,
                                 skip_group_check=True)
                nc.tensor.matmul(
                    cos[:],
                    lbt[:, lb_off:(lb_off + 2 + c):(1 + c), :],
                    ctt[:, (ct_off + c):(ct_off + 5):(4 - c), :],
                    start=False, stop=True, perf_mode=DR,
                    skip_group_check=True)
                nc.scalar.activation(wt8[:, c, :], cos[:], AF.Exp,
                                     scale=beta)
            for c in range(NCH):
                nc.tensor.matmul(num_ps[:], hn8_lhs[:, c, :],
                                 wt8[:, c, :], start=(c == 0),
                                 stop=(num_stop and c == NCH - 1),
                                 skip_group_check=True)
                nc.tensor.matmul(den_ps[0:1, :], ones_col_bf[:],
                                 wt8[:, c, :], start=(c == 0),
                                 stop=(c == NCH - 1),
                                 skip_group_check=True)

        def norm_rows(den_ps, nn):
            """recip (DVE), mask (Pool, from recip), dmm (DVE), dmm bcast
            (Pool). Returns (dmp [128,nn] f32, mask_row bf16)."""
            rr = pring2.tile([1, nn], F32, name="rr", tag="rr")
            nc.vector.reciprocal(rr[:], den_ps[0:1, :])
            mk = pring.tile([1, nn], BF16, name="mk", tag="mk")
            nc.vector.tensor_scalar(mk[:], rr[:], MASK_THR, None,
                                    op0=ALU.is_lt)
            dm = pring2.tile([1, nn], BF16, name="dm", tag="dm")
            nc.vector.tensor_tensor(dm[:], rr[:], mk[:], op=ALU.mult)
            dmp = pring2.tile([128, nn], BF16, name="dmp", tag="dmp")
            nc.gpsimd.partition_broadcast(dmp[:], dm[:])
            return dmp, mk

        # ================= query phase =================
        def query_phase(l):
            if l == 0:
                fm = ps_acc.tile([128, N], F32, name="fm", tag="acc")
                nc.tensor.matmul(fm[:], wq_s[:], xqt_s[:],
                                 start=True, stop=True)
                nc.scalar.activation(HQT[:], fm[:], AF.Identity,
                                     bias=bqc_s[:])
                nm = ps_acc.tile([128, N], F32, name="nm", tag="acc")
                for c in range(NCH):
                    nc.tensor.matmul(nm[:, ts(c, 128)], xqt_s[:, ts(c, 128)],
                                     wq_s[:], start=True, stop=True)
                nc.vector.tensor_copy(
                    HQN[:], nm[:].rearrange("p (c f) -> p c f", c=4))

            sqq = prow1.tile([128, N], BF16, name="sqq", tag="sqq")
            nc.vector.tensor_tensor(sqq[:], HQT[:], HQT[:], op=ALU.mult)
            nsq = ps_row.tile([32, 2 * N], F32, name="nsq", tag="row")
            nc.tensor.matmul(nsq[0:1, 0:N], ones_col_bf[:], sqq[:],
                             start=True, stop=True)
            lnr = prow1.tile([1, N], F32, name="lnrq", tag="lnrq")
            nc.scalar.activation(lnr[:], nsq[0:1, 0:N], AF.Ln, bias=qeps[:])
            irq = prow1.tile([1, N], BF16, name="irq", tag="irq")
            nc.scalar.activation(irq[:], lnr[:], AF.Exp, scale=-0.5)
            irb = prow1.tile([128, N], BF16, name="irbq", tag="irbq")
            nc.gpsimd.partition_broadcast(irb[:], irq[:])
            hnq = prow1.tile([128, N], FP8, name="hnq", tag="hnq")
            nc.vector.tensor_tensor(hnq[:], HQT[:], irb[:], op=ALU.mult)

            wtq = prow1.tile([128, 4, N], BF16, name="wtq", tag="wtq")
            num_ps = ps_acc.tile([128, N], F32, name="numq", tag="acc")
            denq = ps_row.tile([32, 2 * N], F32, name="denq", tag="row")
            den_ps = denq[:, 0:N]
            attention(hnq, 0, lhsbq_t, 0, ctq_t, 0, HQN[:],
                      betq_s[l][:], wtq[:], num_ps, den_ps, num_stop=False)
            # query bias fold: num += k*bq (x) den  (k=2 at l=0, 1 later)
            nc.vector.tensor_copy(den_bfq[:], den_ps[0:1, :])
            nc.tensor.matmul(num_ps[:], bq2row_s[l][:], den_bfq[:],
                             start=False, stop=True, skip_group_check=True)
            dmp, _mk = norm_rows(den_ps, N)
            nc.vector.tensor_tensor(HQT[:], num_ps[:], dmp[:], op=ALU.mult)
            if l < L - 1:
                tp = ps_acc.tile([128, N], F32, name="qtp", tag="acc")
                for c in range(NCH):
                    nc.tensor.transpose(tp[:, ts(c, 128)], HQT[:, ts(c, 128)],
                                        ident_f[:])
                nc.vector.tensor_copy(
                    HQN[:], tp[:].rearrange("p (c f) -> p c f", c=4))

            # per-graph aggregates u, c, v
            uf = pwork.tile([128, G32], F32, name="uf", tag="uf")
            nc.vector.tensor_reduce(
                uf[:], HQT[:].rearrange("p (g k) -> p g k", k=NQPG),
                axis=mybir.AxisListType.X, op=ALU.add)
            nc.vector.tensor_copy(u_bf[:], uf[:])
            squ = pwork.tile([128, G32], BF16, name="squ", tag="uf")
            nc.vector.tensor_tensor(squ[:], uf[:], uf[:], op=ALU.mult)
            crow_ps = ps_row.tile([32, 2 * N], F32, name="crow_ps", tag="row")
            nc.tensor.matmul(crow_ps[0:1, 0:G32], ones_col_bf[:], squ[:],
                             start=True, stop=True)
            nc.vector.tensor_copy(c_row[:], crow_ps[0:1, 0:G32])
            nc.vector.tensor_scalar(ce_row[:], crow_ps[0:1, 0:G32], 1e-24,
                                    None, op0=ALU.add)
            # w = b1t^T u + a1^T b_prev  (the v-mask rank-1 vector)
            wv = ps_acc.tile([128, N], F32, name="wv", tag="acc")
            nc.tensor.matmul(wv[:, 0:G32], b1t_s[l][:], u_bf[:],
                             start=True, stop=True, skip_group_check=True)
            nc.tensor.matmul(wv[:, ds(G32, 1)], a1_s[l][:], bprev_s[l][:],
                             start=True, stop=True, skip_group_check=True)
            nc.vector.tensor_copy(wfold[:], wv[:, ds(G32, 1)])
            wsb = pwork.tile([128, G32], F32, name="wsb", tag="wsb")
            nc.scalar.activation(wsb[:], wv[:, 0:G32], AF.Identity,
                                 bias=wfold[:])
            vt_ps = ps_acc.tile([128, N], F32, name="vtp", tag="acc")
            nc.tensor.transpose(vt_ps[0:G32, 0:128], wsb[:], ident_f[:])
            nc.vector.tensor_copy(vt32[:], vt_ps[0:G32, 0:128])
            nc.sync.dma_start(vrow[:], vt32[:])

        # ================= data pipeline =================
        def prep_dma(l, p):
            gA = 2 * p
            ctp = pct.tile([128, 10, N], FP8, name="ctp", tag="ctp")
            nc.sync.dma_start(
                ctp[:],
                io["ct"][ds(gA, 2)].rearrange("g b p f -> p (g b) f"))
            xgp = None
            if l == 0:
                xgp = pct.tile([IN, 2 * N], BF16, name="xgp", tag="xgp")
                nc.sync.dma_start(xgp[:], io["xt"][:, ds(gA * N, 2 * N)])
            return ctp, xgp

        def prep(l, p, ctp, xgp, skip_proj=False):
            gA = 2 * p
            if l == 0 and not skip_proj:
                for gi in range(2):
                    proj_graph(gA + gi, xgp, gi)
            hTp = HGT[:, ds(gA * N, 2 * N)]
            sq = pmid.tile([128, 2 * N], BF16, name="sq", tag="sq")
            nc.vector.tensor_tensor(sq[:], hTp, hTp, op=ALU.mult)
            lnr = prow.tile([1, 2 * N], F32, name="lnr", tag="lnr")
            nsq = ps_row.tile([32, 2 * N], F32, name="nsq", tag="row")
            for gi in range(2):
                nc.tensor.matmul(nsq[0:1, ds(gi * N, N)], ones_col_bf[:],
                                 sq[:, ds(gi * N, N)], start=True, stop=True)
                nc.scalar.activation(lnr[0:1, ds(gi * N, N)],
                                     nsq[0:1, ds(gi * N, N)], AF.Ln,
                                     bias=ce_row[0:1, ds(gA + gi, 1)])
            irr = prow.tile([1, 2 * N], BF16, name="irr", tag="irr")
            nc.scalar.activation(irr[:], lnr[:], AF.Exp, scale=-0.5)
            # b-rows (ir, fp8) into the ct b-blocks of both graphs
            nc.gpsimd.tensor_scalar(
                ctp[0:1, 4:10:5, :],
                irr[:].rearrange("o (g f) -> o g f", g=2), 1.0, None,
                op0=ALU.mult)
            # a-rows (c*ir, fp8) into lhsb blocks 1..4 of each graph
            for gi in range(2):
                g = gA + gi
                nc.vector.tensor_scalar(
                    lhsb_t[0:1, ds(g * 5 + 1, 4), :],
                    irr[0:1, ds(gi * N, N)].rearrange(
                        "o (c f) -> o c f", c=4),
                    c_row[0:1, ds(g, 1)], None, op0=ALU.mult)
            irb = pmid.tile([128, 2 * N], BF16, name="irb", tag="irb")
            nc.gpsimd.partition_broadcast(irb[:], irr[:])
            hn = pmid.tile([128, 2 * N], FP8, name="hn", tag="hn")
            nc.gpsimd.tensor_mul(hn[:], hTp, irb[:])
            return hn

        def pairA(l, p, hn, ctp):
            gA = 2 * p
            den_ps = ps_row.tile([32, 2 * N], F32, name="den", tag="row")
            out = []
            for gi in range(2):
                g = gA + gi
                wt8 = pwork.tile([128, 4, N], BF16, name="wt8", tag="wt8")
                num_ps = ps_acc.tile([128, N], F32, name="num", tag="acc")
                attention(hn, gi * N, lhsb_t, g * 5, ctp, gi * 5,
                          HGN[:, ds(g * 4, 4), :], betg_s[l][:], wt8[:],
                          num_ps, den_ps[:, ds(gi * N, N)], num_stop=True)
                out.append(num_ps)
            return out, den_ps

        def pairR(l, p, ab):
            nums, den_ps = ab
            dmp, mk = norm_rows(den_ps, 2 * N)
            res = []
            for gi in range(2):
                s1 = pring.tile([128, N], BF16, name="s1", tag="s1")
                nc.vector.tensor_tensor(s1[:], nums[gi][:],
                                        dmp[:, ds(gi * N, N)], op=ALU.mult)
                res.append((s1, mk[0:1, ds(gi * N, N)]))
            return res

        def pairB(l, p, rs):
            gA = 2 * p
            for gi in range(2):
                g = gA + gi
                s1, mk = rs[gi]
                z_ps = ps_acc.tile([128, N], F32, name="z", tag="acc")
                nc.tensor.matmul(z_ps[:], a1_s[l][:], s1[:],
                                 start=True, stop=False,
                                 skip_group_check=True)
                nc.tensor.matmul(z_ps[:], vrow[0:1, ts(g, 128)], mk[:],
                                 start=False, stop=True,
                                 skip_group_check=True)
                rz = pmid.tile([128, N], BF16, name="rz", tag="rz")
                nc.vector.tensor_scalar(rz[:], z_ps[:], b1c_s[l][:], 0.0,
                                        op0=ALU.add, op1=ALU.max)
                h2_ps = ps_acc.tile([128, N], F32, name="h2", tag="acc")
                nc.tensor.matmul(h2_ps[:], w2_s[l][:], rz[:],
                                 start=True, stop=True)
                if l < L - 1:
                    nc.scalar.activation(HGT[:, ts(g, N)], h2_ps[:],
                                         AF.Identity, bias=b2c_s[l][:])
                    nm_ps = ps_acc.tile([128, N], F32, name="nm", tag="acc")
                    for c in range(NCH):
                        nc.tensor.matmul(nm_ps[:, ts(c, 128)],
                                         rz[:, ts(c, 128)], w2_s[l][:],
                                         start=True, stop=True)
                    nc.vector.tensor_copy(
                        HGN[:, ds(g * 4, 4), :],
                        nm_ps[:].rearrange("p (c f) -> p c f", c=4))
                else:
                    nc.vector.tensor_reduce(
                        HGS[:, ds(g, 1)], h2_ps[:],
                        axis=mybir.AxisListType.X, op=ALU.add)

        # ================= schedule =================
        for l in range(L):
            ctps = {q: prep_dma(l, q) for q in (0, 1, 2) if q < NPAIR}
            if l == 0:
                for q in (0, 1, 2):
                    for gi in range(2):
                        proj_graph(2 * q + gi, ctps[q][1], gi)
            query_phase(l)
            preps = {}
            for q in (0, 1):
                ctq_, xgq_ = ctps[q]
                preps[q] = (prep(l, q, ctq_, xgq_, skip_proj=(q <= 2)),
                            ctq_)
            pend = None
            for p in range(NPAIR):
                if p + 3 < NPAIR:
                    ctps[p + 3] = prep_dma(l, p + 3)
                if p + 2 < NPAIR:
                    ctn, xgn = ctps.pop(p + 2)
                    preps[p + 2] = (prep(l, p + 2, ctn, xgn,
                                         skip_proj=(p + 2 <= 2)), ctn)
                hn, ctp = preps.pop(p)
                ab = pairA(l, p, hn, ctp)
                rs = pairR(l, p, ab)
                if pend is not None:
                    pairB(l, p - 1, pend)
                pend = rs
            pairB(l, NPAIR - 1, pend)

        # ---- final predictor ----
        hgs_bf = pwork.tile([128, G32], BF16, name="hgs_bf", tag="uf")
        nc.scalar.activation(hgs_bf[:], HGS[:], AF.Identity,
                             bias=b2x512_s[:])
        z1 = ps_acc.tile([128, N], F32, name="z1", tag="acc")
        nc.tensor.matmul(z1[:, 0:G32], wp1_s[:], hgs_bf[:],
                         start=True, stop=True, skip_group_check=True)
        r1 = pwork.tile([128, G32], BF16, name="r1", tag="uf")
        nc.scalar.activation(r1[:], z1[:, 0:G32], AF.Relu, bias=bp1c_s[:])
        y_ps = ps_row.tile([32, 2 * N], F32, name="y_ps", tag="row")
        nc.tensor.matmul(y_ps[0:1, 0:G32], wp2_s[:], r1[:],
                         start=True, stop=True)
        y_sb = prow1.tile([1, G32], F32, name="y_sb", tag="ysb")
        nc.scalar.activation(y_sb[:], y_ps[0:1, 0:G32], AF.Identity,
                             bias=bp2c_s[:])
        nc.sync.dma_start(io["y"][:], y_sb[:])


# ================= host side =================

def _build_ct_np(src, dst, npb, nblocks):
    blk = src // npb
    s = src - blk * npb
    d = dst - blk * npb
    flat = blk * (npb * npb) + s * npb + d
    cnt = np.bincount(flat, minlength=nblocks * npb * npb)
    return cnt.reshape(nblocks, npb, npb)


def _lnct(counts, beta):
    """[nb, 512, 512] counts -> [nb, 5, 128, 512] fp8 ln(ct)/beta with
    LNZ floor, blocks 0..3 = src chunks, block 4 = zeros (b-block)."""
    nb = counts.shape[0]
    out = np.full((nb, 5, 128, N), 0.0, np.float32)
    with np.errstate(divide="ignore"):
        lv = np.where(counts > 0, np.log(np.maximum(counts, 1)),
                      LNZ * beta).astype(np.float32) / beta
    out[:, 0:4] = lv.reshape(nb, 4, 128, N)
    out[:, 4] = 0.0
    return out.astype(ml_dtypes.float8_e4m3)


_PROG_CACHE = {}
_PROG_LOCK = threading.Lock()


def _get_program(gpc=GPC):
    with _PROG_LOCK:
        if gpc not in _PROG_CACHE:
            _PROG_CACHE[gpc] = build_program(gpc)
        return _PROG_CACHE[gpc]


def _make_in_maps(inputs, gpc=GPC, ncores=NCORES):
    bf = ml_dtypes.bfloat16
    f8 = ml_dtypes.float8_e4m3
    X = np.asarray(inputs["X"], np.float32)
    X_q = np.asarray(inputs["X_q"], np.float32)
    g_src = np.asarray(inputs["g_src"], np.int64)
    g_dst = np.asarray(inputs["g_dst"], np.int64)
    q_src = np.asarray(inputs["q_src"], np.int64)
    q_dst = np.asarray(inputs["q_dst"], np.int64)
    betas_g = np.asarray(inputs["betas_g"], np.float32)
    betas_q = np.asarray(inputs["betas_q"], np.float32)
    assert np.all(betas_g > 0) and np.all(betas_q > 0)
    assert np.allclose(betas_g, betas_g[0]) and np.allclose(betas_q,
                                                            betas_q[0])

    W1r = np.asarray(inputs["W1r"], np.float32)
    bg = np.asarray(inputs["bg"], np.float32)
    bq = np.asarray(inputs["bq"], np.float32)
    b1r = np.asarray(inputs["b1r"], np.float32)
    b2r = np.asarray(inputs["b2r"], np.float32)
    bprev = np.stack([bg, b2r[0]])
    bq2 = np.stack([2.0 * bq, bq]).reshape(L, 1, H)

    # lhsb constant part: per graph block0 = identity, blocks 1..4 zero
    lhsb_one = np.zeros((5, 128, 128), np.float32)
    lhsb_one[0] = np.eye(128, dtype=np.float32)
    lhsb = np.broadcast_to(lhsb_one, (gpc, 5, 128, 128)).astype(f8)
    lhsbq = lhsb_one.astype(f8)

    shared = {
        "wg": np.asarray(inputs["Wg"], np.float32).astype(bf),
        "wq": np.asarray(inputs["Wq"], np.float32).astype(bf),
        "bgc": bg.reshape(H, 1).copy(),
        "bqc": bq.reshape(H, 1).copy(),
        "bq2row": bq2.astype(bf),
        "betg": np.tile(betas_g.reshape(L, 1, 1), (1, H, 1)),
        "betq": np.tile(betas_q.reshape(L, 1, 1), (1, H, 1)),
        "a1": np.ascontiguousarray(W1r[:, :H, :]).astype(bf),
        "b1t": np.ascontiguousarray(W1r[:, H:, :]).astype(bf),
        "w2": np.asarray(inputs["W2r"], np.float32).astype(bf),
        "b1c": b1r.reshape(L, H, 1).copy(),
        "b2c": b2r.reshape(L, H, 1).copy(),
        "bprev_bf": bprev.reshape(L, H, 1).astype(bf),
        "b2x512c": (512.0 * b2r[L - 1]).reshape(H, 1).astype(np.float32),
        "wp1": np.asarray(inputs["Wp1"], np.float32).astype(bf),
        "wp2": np.asarray(inputs["Wp2"], np.float32).astype(bf),
        "bp1c": np.asarray(inputs["bp1"], np.float32).reshape(H, 1).copy(),
        "bp2c": np.asarray(inputs["bp2"], np.float32).reshape(1, 1).copy(),
        "lhsb": lhsb,
        "lhsbq": lhsbq,
    }

    n = gpc * NPG
    nq = gpc * NQPG
    ne = n * 8
    nqe = nq * 8
    in_maps = []
    for cid in range(ncores):
        xc = X[cid * n:(cid + 1) * n]
        xqc = X_q[cid * nq:(cid + 1) * nq]
        gs = g_src[cid * ne:(cid + 1) * ne] - cid * n
        gd = g_dst[cid * ne:(cid + 1) * ne] - cid * n
        qs = q_src[cid * nqe:(cid + 1) * nqe] - cid * nq
        qd = q_dst[cid * nqe:(cid + 1) * nqe] - cid * nq

        ct_g_counts = _build_ct_np(gs, gd, NPG, gpc)     # [gpc, 512, 512]
        ct_q = _build_ct_np(qs, qd, NQPG, gpc)           # [gpc, 16, 16]
        ctq_blk = np.zeros((N, N), np.int64)
        for g in range(gpc):
            ctq_blk[g * NQPG:(g + 1) * NQPG,
                    g * NQPG:(g + 1) * NQPG] = ct_q[g]

        m = dict(shared)
        m["xt"] = np.ascontiguousarray(xc.T).astype(bf)
        xqt = np.zeros((IN, N), np.float32)
        xqt[:, :nq] = xqc.T
        m["xqt"] = xqt.astype(bf)
        m["ct"] = _lnct(ct_g_counts, float(betas_g[0]))
        m["ctq"] = _lnct(ctq_blk[None], float(betas_q[0]))[0]
        in_maps.append(m)
    return in_maps


def run(inputs, trace=False, gpc=GPC):
    nc = _get_program(gpc)
    in_maps = _make_in_maps(inputs, gpc=gpc)
    res = run_bass_kernel_spmd(nc, in_maps, list(range(NCORES)), trace=trace)
    ys = [res.results[c]["y"].reshape(-1) for c in range(NCORES)]
    out = np.concatenate(ys).astype(np.float32).reshape(B, OUT)
    return out, res


def kernel(**inputs) -> np.ndarray:
    out, _ = run(inputs, trace=False)
    return out


# revision 72
# speedup vs baseline: 1.4952x; 1.1732x over previous
"""Trainium2 Bass kernel for nn_CascadeGNN (cascade AGNN over 256 graphs).

Graph-sharded SPMD over 8 NeuronCores (32 data graphs/core + one packed
512-node query block). Dense per-graph AGNN with:
  * fp8 (e4m3) cos matmuls; the count-mask rides in PSUM as ln(ct)/beta
    accumulated via a DoubleRow matmul {identity, rank-1 c*ir x ir}, so
    exp(beta*PSUM) directly yields the masked edge weights W in fp8.
  * fp8 DoubleRow num/den matmuls (2 k-tiles of 128 src nodes each).
  * node-major h produced by extra N=128 matmuls (rz^T W2) instead of
    transposes; biases folded algebraically (b2 x den into the v-mask
    rank-1 term, so node-major h stays unbiased).
  * one activation-function table for the whole kernel (Ln/Exp/Identity/
    Relu all live in natural_log_exp_and_others; the act-table chooser is
    steered there to avoid per-pair table reloads).
  * software-pipelined emission: prep(pair p+1) is emitted before
    main(pair p) so every engine always has ready work queued.
"""

import threading
from contextlib import ExitStack

import numpy as np
import ml_dtypes

import concourse.bass as bass
import concourse.mybir as mybir
import concourse.tile as tile
from concourse import bacc
from concourse.bass import ds, ts
from concourse.bass_utils import run_bass_kernel_spmd
from concourse.hw_specs import get_activation_tables
from concourse.masks import make_identity

BF16 = mybir.dt.bfloat16
F32 = mybir.dt.float32
FP8 = mybir.dt.float8e4
AF = mybir.ActivationFunctionType
ALU = mybir.AluOpType
DR = mybir.MatmulPerfMode.DoubleRow

# problem constants
B = 256
NPG = 512
NQPG = 16
IN, H, L, OUT = 64, 128, 2, 1
NCORES = 8
GPC = B // NCORES          # graphs per core (32)
N = NPG                    # dense block size (512)
NCH = N // 128             # 4 chunks of 128 src nodes
G32 = N // NQPG            # 32 query graphs packed into one 512 block
LNZ = -16.0                # ln-count floor for absent edges
MASK_THR = 100.0           # recip threshold: real den -> recip < ~3

_ACT_KEEP = None


def _patched_tables(arch):
    """First-match table choice would thrash between 'natural_log' and
    'exp_and_others'; strip our funcs from earlier tables so every
    activation resolves to natural_log_exp_and_others (one load)."""
    tabs = get_activation_tables(arch)
    keep = {AF.Ln, AF.Exp, AF.Identity, AF.Relu, AF.Copy, AF.Square,
            AF.Sign, AF.MemsetZero, AF.Abs, AF.Is_finite}
    out = {}
    hit = False
    for name, s in tabs.items():
        if name == "natural_log_exp_and_others":
            out[name] = s
            hit = True
        elif not hit:
            out[name] = s - keep
        else:
            out[name] = s
    return out


def build_program(gpc=GPC):
    nc = bacc.Bacc("TRN2", target_bir_lowering=False, debug=False,
                   num_devices=NCORES)
    io = {}
    n_nodes = gpc * NPG
    io["xt"] = nc.dram_tensor("xt", [IN, n_nodes], BF16,
                              kind="ExternalInput").ap()
    io["xqt"] = nc.dram_tensor("xqt", [IN, N], BF16,
                               kind="ExternalInput").ap()
    io["ct"] = nc.dram_tensor("ct", [gpc, 5, 128, N], FP8,
                              kind="ExternalInput").ap()
    io["ctq"] = nc.dram_tensor("ctq", [5, 128, N], FP8,
                               kind="ExternalInput").ap()
    io["lhsb"] = nc.dram_tensor("lhsb", [gpc, 5, 128, 128], FP8,
                                kind="ExternalInput").ap()
    io["lhsbq"] = nc.dram_tensor("lhsbq", [5, 128, 128], FP8,
                                 kind="ExternalInput").ap()
    for nm, shp, dt in [
        ("wg", [IN, H], BF16), ("wq", [IN, H], BF16),
        ("bgc", [H, 1], F32), ("bqc", [H, 1], F32),
        ("bq2row", [L, 1, H], BF16),          # (2*bq, bq) per layer
        ("betg", [L, H, 1], F32), ("betq", [L, H, 1], F32),
        ("a1", [L, H, H], BF16), ("b1t", [L, H, H], BF16),
        ("w2", [L, H, H], BF16),
        ("b1c", [L, H, 1], F32), ("b2c", [L, H, 1], F32),
        ("bprev_bf", [L, H, 1], BF16),        # (bg, b2r[0]) per layer
        ("wp1", [H, H], BF16), ("wp2", [H, 1], BF16),
        ("b2x512c", [H, 1], F32),
        ("bp1c", [H, 1], F32), ("bp2c", [1, 1], F32),
    ]:
        io[nm] = nc.dram_tensor(nm, shp, dt, kind="ExternalInput").ap()
    io["y"] = nc.dram_tensor("y", [1, gpc], F32, kind="ExternalOutput").ap()

    orig = bacc.get_activation_tables
    bacc.get_activation_tables = _patched_tables
    try:
        with tile.TileContext(nc) as tc:
            _emit(tc, nc, gpc, io)
        nc.compile()
    finally:
        bacc.get_activation_tables = orig
    return nc


def _emit(tc, nc, gpc, io):
    n_nodes = gpc * NPG
    NPAIR = gpc // 2

    ctx = ExitStack()
    with ctx:
        pconst = ctx.enter_context(tc.tile_pool(name="pconst", bufs=1))
        pstate = ctx.enter_context(tc.tile_pool(name="pstate", bufs=1))
        pwork = ctx.enter_context(tc.tile_pool(name="pwork", bufs=2))
        pct = ctx.enter_context(tc.tile_pool(name="pct", bufs=3))
        prow = ctx.enter_context(tc.tile_pool(name="prow", bufs=3))
        prow1 = ctx.enter_context(tc.tile_pool(name="prow1", bufs=1))
        pring = ctx.enter_context(tc.tile_pool(name="pring", bufs=4))
        pring2 = ctx.enter_context(tc.tile_pool(name="pring2", bufs=2))
        pmid = ctx.enter_context(tc.tile_pool(name="pmid", bufs=3))
        ps_cos = ctx.enter_context(
            tc.tile_pool(name="ps_cos", bufs=3, space="PSUM"))
        ps_acc = ctx.enter_context(
            tc.tile_pool(name="ps_acc", bufs=3, space="PSUM"))
        ps_row = ctx.enter_context(
            tc.tile_pool(name="ps_row", bufs=1, space="PSUM"))

        def const(name, shape, dtype):
            return pconst.tile(shape, dtype, name=name, tag=name)

        # ---- constants ----
        ident_f = const("ident_f", [128, 128], F32)
        make_identity(nc, ident_f[:])
        ones_col_bf = const("ones_col_bf", [128, 1], BF16)
        nc.vector.memset(ones_col_bf[:], 1.0)
        qeps = const("qeps", [1, 1], F32)
        nc.vector.memset(qeps[:], 1e-24)

        # ---- weights ----
        def load(name, ap_dram, shape, dtype):
            t = const(name, shape, dtype)
            nc.sync.dma_start(t[:], ap_dram)
            return t

        wg_s = load("wg_s", io["wg"][:], [IN, H], BF16)
        wq_s = load("wq_s", io["wq"][:], [IN, H], BF16)
        bgc_s = load("bgc_s", io["bgc"][:], [H, 1], F32)
        bqc_s = load("bqc_s", io["bqc"][:], [H, 1], F32)
        betq_s = [load(f"betq{l}", io["betq"][l], [H, 1], F32)
                  for l in range(L)]
        xqt_s = load("xqt_s", io["xqt"][:], [IN, N], BF16)
        ctq_t = load("ctq_t", io["ctq"][:].rearrange("b p f -> p b f"),
                     [128, 5, N], FP8)
        lhsbq_t = load("lhsbq_t",
                       io["lhsbq"][:].rearrange("b p f -> p b f"),
                       [128, 5, 128], FP8)
        a1_s, b1t_s, w2_s, b1c_s, b2c_s, bprev_s = [], [], [], [], [], []
        bq2row_s, betg_s = [], []
        late = {}

        def load_rest():
            for l in range(L):
                betg_s.append(load(f"betg{l}", io["betg"][l], [H, 1], F32))
                bq2row_s.append(load(f"bq2row{l}", io["bq2row"][l],
                                     [1, H], BF16))
            for l in range(L):
                a1_s.append(load(f"a1_{l}", io["a1"][l], [H, H], BF16))
                b1t_s.append(load(f"b1t_{l}", io["b1t"][l], [H, H], BF16))
                bprev_s.append(load(f"bprev{l}", io["bprev_bf"][l],
                                    [H, 1], BF16))
            late["lhsb_t"] = load(
                "lhsb_t", io["lhsb"][:].rearrange("g b p f -> p (g b) f"),
                [128, gpc * 5, 128], FP8)
            for l in range(L):
                w2_s.append(load(f"w2_{l}", io["w2"][l], [H, H], BF16))
                b1c_s.append(load(f"b1c_{l}", io["b1c"][l], [H, 1], F32))
                b2c_s.append(load(f"b2c_{l}", io["b2c"][l], [H, 1], F32))
            late["wp1_s"] = load("wp1_s", io["wp1"][:], [H, H], BF16)
            late["wp2_s"] = load("wp2_s", io["wp2"][:], [H, 1], BF16)
            late["bp1c_s"] = load("bp1c_s", io["bp1c"][:], [H, 1], F32)
            late["b2x512_s"] = load("b2x512_s", io["b2x512c"][:],
                                    [H, 1], F32)
            late["bp2c_s"] = load("bp2c_s", io["bp2c"][:], [1, 1], F32)

        # ---- persistent state ----
        def state(name, shape, dtype):
            return pstate.tile(shape, dtype, name=name, tag=name)

        HGT = state("HGT", [128, n_nodes], BF16)          # h_g feature-major
        HGN = state("HGN", [128, gpc * 4, 128], BF16)     # node-major, no b2
        HQT = state("HQT", [128, N], F32)
        HQN = state("HQN", [128, 4, 128], BF16)
        HGS = state("HGS", [128, gpc], F32)
        u_bf = [state(f"u_bf{l}", [128, G32], BF16) for l in range(L)]
        c_row = [state(f"c_row{l}", [1, G32], F32) for l in range(L)]
        ce_row = [state(f"ce_row{l}", [1, G32], F32) for l in range(L)]
        vt32 = [state(f"vt32_{l}", [G32, 128], BF16) for l in range(L)]
        vrow = [state(f"vrow{l}", [1, G32 * 128], BF16) for l in range(L)]
        wfold = [state(f"wfold{l}", [128, 1], F32) for l in range(L)]


        def lb_g(g):
            return lhsb_t[:, ds(g * 5, 5), :]

        # ================= helpers =================
        def proj_graph(g, xgp, gi):
            """Initial projection for data graph g: feature-major (bias bg
            via ACT) + node-major (no bias) into HGN. xgp: [IN, 2N] pair."""
            fm = ps_cos.tile([128, N], F32, name="fm", tag="cos")
            nc.tensor.matmul(fm[:], wg_s[:], xgp[:, ds(gi * N, N)],
                             start=True, stop=True)
            nc.scalar.activation(HGT[:, ts(g, N)], fm[:], AF.Identity,
                                 bias=bgc_s[:])
            nm = ps_cos.tile([128, N], F32, name="nm", tag="cos")
            for c in range(NCH):
                nc.tensor.matmul(nm[:, ts(c, 128)],
                                 xgp[:, ds(gi * N + c * 128, 128)],
                                 wg_s[:], start=True, stop=True)
            nc.vector.tensor_copy(HGN[:, ds(g * 4, 4), :],
                                  nm[:].rearrange("p (c f) -> p c f", c=4))

        def attention_cos(hn_tile, hn_off, lbt, lb_off, ctt, ct_off,
                          beta, wt8):
            for c in range(NCH):
                cos = ps_cos.tile([128, N], F32, name="cos", tag="cos")
                nc.tensor.matmul(cos[:],
                                 hn_tile[:, ds(hn_off + c * 128, 128)],
                                 hn_tile[:, ds(hn_off, N)],
                                 start=True, stop=False,
                                 skip_group_check=True)
                nc.tensor.matmul(
                    cos[:],
                    lbt[:, lb_off:(lb_off + 2 + c):(1 + c), :],
                    ctt[:, (ct_off + c):(ct_off + 5):(4 - c), :],
                    start=False, stop=True, perf_mode=DR,
                    skip_group_check=True)
                nc.scalar.activation(wt8[:, c, :], cos[:], AF.Exp,
                                     scale=beta)

        def attention_numden(hn8_lhs, wt8, num_ps, den_ps, num_stop):
            for c in range(NCH):
                nc.tensor.matmul(num_ps[:], hn8_lhs[:, c, :],
                                 wt8[:, c, :], start=(c == 0),
                                 stop=(num_stop and c == NCH - 1),
                                 skip_group_check=True)
                nc.tensor.matmul(den_ps[0:1, :], ones_col_bf[:],
                                 wt8[:, c, :], start=(c == 0),
                                 stop=(c == NCH - 1),
                                 skip_group_check=True)

        def attention(hn_tile, hn_off, lbt, lb_off, ctt, ct_off, hn8_lhs,
                      beta, wt8, num_ps, den_ps, num_stop):
            attention_cos(hn_tile, hn_off, lbt, lb_off, ctt, ct_off,
                          beta, wt8)
            attention_numden(hn8_lhs, wt8, num_ps, den_ps, num_stop)

        def norm_rows(den_ps, nn):
            """recip (DVE), mask (Pool, from recip), dmm (DVE), dmm bcast
            (Pool). Returns (dmp [128,nn] f32, mask_row bf16)."""
            rr = pring2.tile([1, nn], F32, name="rr", tag="rr")
            nc.vector.reciprocal(rr[:], den_ps[0:1, :])
            mk = pring.tile([1, nn], BF16, name="mk", tag="mk")
            nc.vector.tensor_scalar(mk[:], rr[:], MASK_THR, None,
                                    op0=ALU.is_lt)
            dm = pring2.tile([1, nn], BF16, name="dm", tag="dm")
            nc.vector.tensor_tensor(dm[:], rr[:], mk[:], op=ALU.mult)
            dmp = pring2.tile([128, nn], BF16, name="dmp", tag="dmp")
            nc.gpsimd.partition_broadcast(dmp[:], dm[:])
            return dmp, mk

        # ================= query phase =================
        def query_phase_a(l):
            if l == 0:
                fm = ps_acc.tile([128, N], F32, name="fm", tag="acc")
                nc.tensor.matmul(fm[:], wq_s[:], xqt_s[:],
                                 start=True, stop=True)
                nc.scalar.activation(HQT[:], fm[:], AF.Identity,
                                     bias=bqc_s[:])
                nm = ps_acc.tile([128, N], F32, name="nm", tag="acc")
                for c in range(NCH):
                    nc.tensor.matmul(nm[:, ts(c, 128)], xqt_s[:, ts(c, 128)],
                                     wq_s[:], start=True, stop=True)
                nc.vector.tensor_copy(
                    HQN[:], nm[:].rearrange("p (c f) -> p c f", c=4))

            sqq = prow1.tile([128, N], BF16, name="sqq", tag="sqq")
            nc.vector.tensor_tensor(sqq[:], HQT[:], HQT[:], op=ALU.mult)
            nsq = ps_row.tile([32, 2 * N], F32, name="nsq", tag="row")
            nc.tensor.matmul(nsq[0:1, 0:N], ones_col_bf[:], sqq[:],
                             start=True, stop=True)
            lnr = prow1.tile([1, N], F32, name="lnrq", tag="lnrq")
            nc.scalar.activation(lnr[:], nsq[0:1, 0:N], AF.Ln, bias=qeps[:])
            irq = prow1.tile([1, N], BF16, name="irq", tag="irq")
            nc.scalar.activation(irq[:], lnr[:], AF.Exp, scale=-0.5)
            irb = prow1.tile([128, N], BF16, name="irbq", tag="irbq")
            nc.gpsimd.partition_broadcast(irb[:], irq[:])
            hnq = prow1.tile([128, N], FP8, name="hnq", tag="hnq")
            nc.vector.tensor_tensor(hnq[:], HQT[:], irb[:], op=ALU.mult)

            wtq = prow1.tile([128, 4, N], BF16, name="wtq", tag="wtq")
            attention_cos(hnq, 0, lhsbq_t, 0, ctq_t, 0, betq_s[l][:],
                          wtq[:])
            return wtq

        def query_phase_b(l, qa):
            wtq = qa
            num_ps = ps_acc.tile([128, N], F32, name="numq", tag="acc")
            denq = ps_row.tile([32, 2 * N], F32, name="denq", tag="row")
            den_ps = denq[:, 0:N]
            attention_numden(HQN[:], wtq[:], num_ps, den_ps,
                             num_stop=False)
            # query bias fold: num += k*bq (x) den  (k=2 at l=0, 1 later)
            den_bfq = prow1.tile([1, N], BF16, name="den_bfq", tag="den_bfq")
            nc.vector.tensor_copy(den_bfq[:], den_ps[0:1, :])
            nc.tensor.matmul(num_ps[:], bq2row_s[l][:], den_bfq[:],
                             start=False, stop=True, skip_group_check=True)
            dmp, _mk = norm_rows(den_ps, N)
            nc.vector.tensor_tensor(HQT[:], num_ps[:], dmp[:], op=ALU.mult)
            if l < L - 1:
                tp = ps_acc.tile([128, N], F32, name="qtp", tag="acc")
                for c in range(NCH):
                    nc.tensor.transpose(tp[:, ts(c, 128)], HQT[:, ts(c, 128)],
                                        ident_f[:])
                nc.vector.tensor_copy(
                    HQN[:], tp[:].rearrange("p (c f) -> p c f", c=4))

        def query_phase_c(l):
            # per-graph aggregates u, c, v
            uf = pwork.tile([128, G32], F32, name="uf", tag="uf")
            nc.vector.tensor_reduce(
                uf[:], HQT[:].rearrange("p (g k) -> p g k", k=NQPG),
                axis=mybir.AxisListType.X, op=ALU.add)
            nc.vector.tensor_copy(u_bf[l][:], uf[:])
            squ = pwork.tile([128, G32], BF16, name="squ", tag="uf")
            nc.vector.tensor_tensor(squ[:], uf[:], uf[:], op=ALU.mult)
            crow_ps = ps_row.tile([32, 2 * N], F32, name="crow_ps", tag="row")
            nc.tensor.matmul(crow_ps[0:1, 0:G32], ones_col_bf[:], squ[:],
                             start=True, stop=True)
            nc.vector.tensor_copy(c_row[l][:], crow_ps[0:1, 0:G32])
            nc.vector.tensor_scalar(ce_row[l][:], crow_ps[0:1, 0:G32], 1e-24,
                                    None, op0=ALU.add)
            # w = b1t^T u + a1^T b_prev  (the v-mask rank-1 vector)
            wv = ps_acc.tile([128, N], F32, name="wv", tag="acc")
            nc.tensor.matmul(wv[:, 0:G32], b1t_s[l][:], u_bf[l][:],
                             start=True, stop=True, skip_group_check=True)
            nc.tensor.matmul(wv[:, ds(G32, 1)], a1_s[l][:], bprev_s[l][:],
                             start=True, stop=True, skip_group_check=True)
            nc.vector.tensor_copy(wfold[l][:], wv[:, ds(G32, 1)])
            wsb = pwork.tile([128, G32], F32, name="wsb", tag="wsb")
            nc.scalar.activation(wsb[:], wv[:, 0:G32], AF.Identity,
                                 bias=wfold[l][:])
            vt_ps = ps_acc.tile([128, N], F32, name="vtp", tag="acc")
            nc.tensor.transpose(vt_ps[0:G32, 0:128], wsb[:], ident_f[:])
            nc.vector.tensor_copy(vt32[l][:], vt_ps[0:G32, 0:128])
            nc.sync.dma_start(vrow[l][:], vt32[l][:])

        def query_phase(l):
            qa = query_phase_a(l)
            query_phase_b(l, qa)
            query_phase_c(l)

        # ================= data pipeline =================
        def prep_dma(l, p):
            gA = 2 * p
            ctp = pct.tile([128, 10, N], FP8, name="ctp", tag="ctp")
            nc.sync.dma_start(
                ctp[:],
                io["ct"][ds(gA, 2)].rearrange("g b p f -> p (g b) f"))
            xgp = None
            if l == 0:
                xgp = pct.tile([IN, 2 * N], BF16, name="xgp", tag="xgp")
                nc.sync.dma_start(xgp[:], io["xt"][:, ds(gA * N, 2 * N)])
            return ctp, xgp

        def prep(l, p, ctp, xgp, skip_proj=False):
            gA = 2 * p
            if l == 0 and not skip_proj:
                for gi in range(2):
                    proj_graph(gA + gi, xgp, gi)
            hTp = HGT[:, ds(gA * N, 2 * N)]
            sq = pwork.tile([128, 2 * N], BF16, name="sq", tag="sq")
            nc.vector.tensor_tensor(sq[:], hTp, hTp, op=ALU.mult)
            lnr = prow1.tile([1, 2 * N], F32, name="lnr", tag="lnr")
            nsq = ps_row.tile([32, 2 * N], F32, name="nsq", tag="row")
            for gi in range(2):
                nc.tensor.matmul(nsq[0:1, ds(gi * N, N)], ones_col_bf[:],
                                 sq[:, ds(gi * N, N)], start=True, stop=True)
                nc.scalar.activation(lnr[0:1, ds(gi * N, N)],
                                     nsq[0:1, ds(gi * N, N)], AF.Ln,
                                     bias=ce_row[l][0:1, ds(gA + gi, 1)])
            irr = prow.tile([1, 2 * N], BF16, name="irr", tag="irr")
            nc.scalar.activation(irr[:], lnr[:], AF.Exp, scale=-0.5)
            # b-rows (ir, fp8) into the ct b-blocks of both graphs
            nc.gpsimd.tensor_scalar(
                ctp[0:1, 4:10:5, :],
                irr[:].rearrange("o (g f) -> o g f", g=2), 1.0, None,
                op0=ALU.mult)
            # a-rows (c*ir, fp8) into lhsb blocks 1..4 of each graph
            for gi in range(2):
                g = gA + gi
                nc.vector.tensor_scalar(
                    late["lhsb_t"][0:1, ds(g * 5 + 1, 4), :],
                    irr[0:1, ds(gi * N, N)].rearrange(
                        "o (c f) -> o c f", c=4),
                    c_row[l][0:1, ds(g, 1)], None, op0=ALU.mult)
            irb = pmid.tile([128, 2 * N], BF16, name="irb", tag="irb")
            nc.gpsimd.partition_broadcast(irb[:], irr[:])
            hn = pmid.tile([128, 2 * N], FP8, name="hn", tag="hn")
            nc.gpsimd.tensor_mul(hn[:], hTp, irb[:])
            return hn

        def pairA(l, p, hn, ctp):
            gA = 2 * p
            den_ps = ps_row.tile([32, 2 * N], F32, name="den", tag="row")
            out = []
            for gi in range(2):
                g = gA + gi
                wt8 = pwork.tile([128, 4, N], BF16, name="wt8", tag="wt8")
                num_ps = ps_acc.tile([128, N], F32, name="num", tag="acc")
                attention(hn, gi * N, late["lhsb_t"], g * 5, ctp, gi * 5,
                          HGN[:, ds(g * 4, 4), :], betg_s[l][:], wt8[:],
                          num_ps, den_ps[:, ds(gi * N, N)], num_stop=True)
                out.append(num_ps)
            return out, den_ps

        def pairR(l, p, ab):
            nums, den_ps = ab
            dmp, mk = norm_rows(den_ps, 2 * N)
            res = []
            for gi in range(2):
                s1 = pring.tile([128, N], BF16, name="s1", tag="s1")
                nc.vector.tensor_tensor(s1[:], nums[gi][:],
                                        dmp[:, ds(gi * N, N)], op=ALU.mult)
                res.append((s1, mk[0:1, ds(gi * N, N)]))
            return res

        def pairB(l, p, rs):
            gA = 2 * p
            zs = []
            for gi in range(2):
                g = gA + gi
                s1, mk = rs[gi]
                z_ps = ps_acc.tile([128, N], F32, name="z", tag="acc")
                nc.tensor.matmul(z_ps[:], a1_s[l][:], s1[:],
                                 start=True, stop=False,
                                 skip_group_check=True)
                nc.tensor.matmul(z_ps[:], vrow[l][0:1, ts(g, 128)], mk[:],
                                 start=False, stop=True,
                                 skip_group_check=True)
                zs.append(z_ps)
            rzs = []
            for gi in range(2):
                rz = pmid.tile([128, N], BF16, name="rz", tag="rz")
                nc.vector.tensor_scalar(rz[:], zs[gi][:], b1c_s[l][:], 0.0,
                                        op0=ALU.add, op1=ALU.max)
                rzs.append(rz)
            for gi in range(2):
                g = gA + gi
                rz = rzs[gi]
                if l < L - 1:
                    h2_ps = ps_acc.tile([128, N], F32, name="h2", tag="acc")
                    nc.tensor.matmul(h2_ps[:], w2_s[l][:], rz[:],
                                     start=True, stop=True)
                    nc.scalar.activation(HGT[:, ts(g, N)], h2_ps[:],
                                         AF.Identity, bias=b2c_s[l][:])
                    nm_ps = ps_acc.tile([128, N], F32, name="nm", tag="acc")
                    for c in range(NCH):
                        nc.tensor.matmul(nm_ps[:, ts(c, 128)],
                                         rz[:, ts(c, 128)], w2_s[l][:],
                                         start=True, stop=True)
                    nc.vector.tensor_copy(
                        HGN[:, ds(g * 4, 4), :],
                        nm_ps[:].rearrange("p (c f) -> p c f", c=4))
                else:
                    # hg_sum = w2^T (sum_n rz) + 512*b2, applied at predictor
                    nc.vector.tensor_reduce(
                        HGS[:, ds(g, 1)], rz[:],
                        axis=mybir.AxisListType.X, op=ALU.add)

        # ================= schedule =================
        first = True
        for l in range(L):
            ctps = {q: prep_dma(l, q) for q in (0, 1, 2) if q < NPAIR}
            if first:
                load_rest()
                qa0 = query_phase_a(0)
                for gi in range(2):
                    proj_graph(gi, ctps[0][1], gi)
                query_phase_b(0, qa0)
                for gi in range(2):
                    proj_graph(2 + gi, ctps[1][1], gi)
                for gi in range(2):
                    proj_graph(4 + gi, ctps[2][1], gi)
                query_phase_c(0)
                first = False
            preps = {}
            for q in (0, 1):
                ctq_, xgq_ = ctps[q]
                preps[q] = (prep(l, q, ctq_, xgq_, skip_proj=True), ctq_)
            pend = None
            for p in range(NPAIR):
                if p + 3 < NPAIR:
                    ctps[p + 3] = prep_dma(l, p + 3)
                if p + 2 < NPAIR:
                    ctn, xgn = ctps.pop(p + 2)
                    preps[p + 2] = (prep(l, p + 2, ctn, xgn,
                                         skip_proj=(l == 0 and p + 2 == 2)),
                                    ctn)
                hn, ctp = preps.pop(p)
                ab = pairA(l, p, hn, ctp)
                rs = pairR(l, p, ab)
                if l == 0 and p == 2:
                    qa1 = query_phase_a(1)
                if l == 0 and p == 4:
                    query_phase_b(1, qa1)
                if l == 0 and p == 6:
                    query_phase_c(1)
                if pend is not None:
                    pairB(l, p - 1, pend)
                pend = rs
            pairB(l, NPAIR - 1, pend)

        # ---- final predictor ----
        rzs_bf = pwork.tile([128, G32], BF16, name="rzs_bf", tag="uf")
        nc.vector.tensor_copy(rzs_bf[:], HGS[:])
        hg_ps = ps_acc.tile([128, N], F32, name="hgps", tag="acc")
        nc.tensor.matmul(hg_ps[:, 0:G32], w2_s[L - 1][:], rzs_bf[:],
                         start=True, stop=True, skip_group_check=True)
        hgs_bf = pwork.tile([128, G32], BF16, name="hgs_bf", tag="uf")
        nc.scalar.activation(hgs_bf[:], hg_ps[:, 0:G32], AF.Identity,
                             bias=late["b2x512_s"][:])
        z1 = ps_acc.tile([128, N], F32, name="z1", tag="acc")
        nc.tensor.matmul(z1[:, 0:G32], late["wp1_s"][:], hgs_bf[:],
                         start=True, stop=True, skip_group_check=True)
        r1 = pwork.tile([128, G32], BF16, name="r1", tag="uf")
        nc.scalar.activation(r1[:], z1[:, 0:G32], AF.Relu,
                             bias=late["bp1c_s"][:])
        y_ps = ps_row.tile([32, 2 * N], F32, name="y_ps", tag="row")
        nc.tensor.matmul(y_ps[0:1, 0:G32], late["wp2_s"][:], r1[:],
                         start=True, stop=True)
        y_sb = prow1.tile([1, G32], F32, name="y_sb", tag="ysb")
        nc.scalar.activation(y_sb[:], y_ps[0:1, 0:G32], AF.Identity,
                             bias=late["bp2c_s"][:])
        nc.sync.dma_start(io["y"][:], y_sb[:])


# ================= host side =================

def _build_ct_np(src, dst, npb, nblocks):
    blk = src // npb
    s = src - blk * npb
    d = dst - blk * npb
    flat = blk * (npb * npb) + s * npb + d
    cnt = np.bincount(flat, minlength=nblocks * npb * npb)
    return cnt.reshape(nblocks, npb, npb)


def _lnct(counts, beta):
    """[nb, 512, 512] counts -> [nb, 5, 128, 512] fp8 ln(ct)/beta with
    LNZ floor, blocks 0..3 = src chunks, block 4 = zeros (b-block)."""
    nb = counts.shape[0]
    out = np.full((nb, 5, 128, N), 0.0, np.float32)
    with np.errstate(divide="ignore"):
        lv = np.where(counts > 0, np.log(np.maximum(counts, 1)),
                      LNZ * beta).astype(np.float32) / beta
    out[:, 0:4] = lv.reshape(nb, 4, 128, N)
    out[:, 4] = 0.0
    return out.astype(ml_dtypes.float8_e4m3)


_PROG_CACHE = {}
_PROG_LOCK = threading.Lock()


def _get_program(gpc=GPC):
    with _PROG_LOCK:
        if gpc not in _PROG_CACHE:
            _PROG_CACHE[gpc] = build_program(gpc)
        return _PROG_CACHE[gpc]


def _make_in_maps(inputs, gpc=GPC, ncores=NCORES):
    bf = ml_dtypes.bfloat16
    f8 = ml_dtypes.float8_e4m3
    X = np.asarray(inputs["X"], np.float32)
    X_q = np.asarray(inputs["X_q"], np.float32)
    g_src = np.asarray(inputs["g_src"], np.int64)
    g_dst = np.asarray(inputs["g_dst"], np.int64)
    q_src = np.asarray(inputs["q_src"], np.int64)
    q_dst = np.asarray(inputs["q_dst"], np.int64)
    betas_g = np.asarray(inputs["betas_g"], np.float32)
    betas_q = np.asarray(inputs["betas_q"], np.float32)
    assert np.all(betas_g > 0) and np.all(betas_q > 0)
    assert np.allclose(betas_g, betas_g[0]) and np.allclose(betas_q,
                                                            betas_q[0])

    W1r = np.asarray(inputs["W1r"], np.float32)
    bg = np.asarray(inputs["bg"], np.float32)
    bq = np.asarray(inputs["bq"], np.float32)
    b1r = np.asarray(inputs["b1r"], np.float32)
    b2r = np.asarray(inputs["b2r"], np.float32)
    bprev = np.stack([bg, b2r[0]])
    bq2 = np.stack([2.0 * bq, bq]).reshape(L, 1, H)

    # lhsb constant part: per graph block0 = identity, blocks 1..4 zero
    lhsb_one = np.zeros((5, 128, 128), np.float32)
    lhsb_one[0] = np.eye(128, dtype=np.float32)
    lhsb = np.broadcast_to(lhsb_one, (gpc, 5, 128, 128)).astype(f8)
    lhsbq = lhsb_one.astype(f8)

    shared = {
        "wg": np.asarray(inputs["Wg"], np.float32).astype(bf),
        "wq": np.asarray(inputs["Wq"], np.float32).astype(bf),
        "bgc": bg.reshape(H, 1).copy(),
        "bqc": bq.reshape(H, 1).copy(),
        "bq2row": bq2.astype(bf),
        "betg": np.tile(betas_g.reshape(L, 1, 1), (1, H, 1)),
        "betq": np.tile(betas_q.reshape(L, 1, 1), (1, H, 1)),
        "a1": np.ascontiguousarray(W1r[:, :H, :]).astype(bf),
        "b1t": np.ascontiguousarray(W1r[:, H:, :]).astype(bf),
        "w2": np.asarray(inputs["W2r"], np.float32).astype(bf),
        "b1c": b1r.reshape(L, H, 1).copy(),
        "b2c": b2r.reshape(L, H, 1).copy(),
        "bprev_bf": bprev.reshape(L, H, 1).astype(bf),
        "b2x512c": (512.0 * b2r[L - 1]).reshape(H, 1).astype(np.float32),
        "wp1": np.asarray(inputs["Wp1"], np.float32).astype(bf),
        "wp2": np.asarray(inputs["Wp2"], np.float32).astype(bf),
        "bp1c": np.asarray(inputs["bp1"], np.float32).reshape(H, 1).copy(),
        "bp2c": np.asarray(inputs["bp2"], np.float32).reshape(1, 1).copy(),
        "lhsb": lhsb,
        "lhsbq": lhsbq,
    }

    n = gpc * NPG
    nq = gpc * NQPG
    ne = n * 8
    nqe = nq * 8
    in_maps = []
    for cid in range(ncores):
        xc = X[cid * n:(cid + 1) * n]
        xqc = X_q[cid * nq:(cid + 1) * nq]
        gs = g_src[cid * ne:(cid + 1) * ne] - cid * n
        gd = g_dst[cid * ne:(cid + 1) * ne] - cid * n
        qs = q_src[cid * nqe:(cid + 1) * nqe] - cid * nq
        qd = q_dst[cid * nqe:(cid + 1) * nqe] - cid * nq

        ct_g_counts = _build_ct_np(gs, gd, NPG, gpc)     # [gpc, 512, 512]
        ct_q = _build_ct_np(qs, qd, NQPG, gpc)           # [gpc, 16, 16]
        ctq_blk = np.zeros((N, N), np.int64)
        for g in range(gpc):
            ctq_blk[g * NQPG:(g + 1) * NQPG,
                    g * NQPG:(g + 1) * NQPG] = ct_q[g]

        m = dict(shared)
        m["xt"] = np.ascontiguousarray(xc.T).astype(bf)
        xqt = np.zeros((IN, N), np.float32)
        xqt[:, :nq] = xqc.T
        m["xqt"] = xqt.astype(bf)
        m["ct"] = _lnct(ct_g_counts, float(betas_g[0]))
        m["ctq"] = _lnct(ctq_blk[None], float(betas_q[0]))[0]
        in_maps.append(m)
    return in_maps


def run(inputs, trace=False, gpc=GPC):
    nc = _get_program(gpc)
    in_maps = _make_in_maps(inputs, gpc=gpc)
    res = run_bass_kernel_spmd(nc, in_maps, list(range(NCORES)), trace=trace)
    ys = [res.results[c]["y"].reshape(-1) for c in range(NCORES)]
    out = np.concatenate(ys).astype(np.float32).reshape(B, OUT)
    return out, res


def kernel(**inputs) -> np.ndarray:
    out, _ = run(inputs, trace=False)
    return out
